# revision 19
# baseline (speedup 1.0000x reference)
"""Trainium2 Bass kernel for an ODE-RNN encoder (z0 posterior).

Model: 128-step reversed-time GRU-like recurrence with an Euler ODE step on
the mean channel, then a final transform producing (mean_z0, std_z0).

Strategy: data-parallel over the subject (batch) dim across 8 NeuronCores,
weights replicated.  Everything runs on-chip in a transposed layout
([feature, batch], batch=256 on the free dim).  Key tricks:
- all biases are added on the TensorEngine (K=1 matmuls against a ones row,
  or a ones row appended to the streamed x tile), so ACT ops never need
  per-half bias vectors;
- sigmoid(z) = 0.5 + 0.5*tanh(z/2) keeps every transcendental in the
  resident `exp_and_others` ACT table set (no per-step table switches);
- softplus(x) = log1p(exp(x)) is computed as Newton iterations on
  y: e^y = 1 + e^x with y0 = relu(x) + ln2*exp(-|x|)  (validated to
  ~1.4e-6 rel error vs the fp32 reference with 2 iterations);
- the update gate, the observation mask (broadcast via an all-ones K=32
  matmul) and the GRU convex combination are folded into a single factor
  G = 0.5*m*(1-tanh(zU/2)):  new = old + G*(cand - old).
"""
import sys
import numpy as np

for _p in ("/opt/trn_rl_repo", "/root/.axon_site/_ro/trn_rl_repo"):
    if _p not in sys.path:
        sys.path.append(_p)

N_SUBJ, N_TP, INPUT_DIM, LATENT, N_UNIT = 2048, 128, 64, 128, 256
HALF = INPUT_DIM // 2
N_CORES = 8
B = N_SUBJ // N_CORES          # 256 subjects per core (free dim)
L = LATENT
SP_ITERS = 2                   # softplus Newton refinements
LN2 = float(np.log(2.0))

_CACHE = {}


# --------------------------------------------------------------------------
# Bass program
# --------------------------------------------------------------------------
def _build(n_tp, sp_iters):
    import concourse.mybir as mybir
    from concourse import bacc, tile

    F32 = mybir.dt.float32
    AF = mybir.ActivationFunctionType
    OP = mybir.AluOpType

    # Bacc (not plain Bass): its compile() runs move_matmul_waits_to_ldweights
    # + generate_event_semaphores, which legalize the TRN2 one-sync-wait-per-
    # instruction limit that walrus enforces.
    nc = bacc.Bacc(None)

    # ---- DRAM I/O ----
    d_x = nc.dram_tensor("x_rev", [n_tp, INPUT_DIM, B], F32, kind="ExternalInput")
    d_dt = nc.dram_tensor("dt_b", [128, n_tp], F32, kind="ExternalInput")

    wspec = {
        # layer-1 K-tiles; kx is [w_x_rows ; bias] (ones row appended to x tile)
        "ug1_k0": [L, N_UNIT], "ug1_k1": [L, N_UNIT], "ug1_kx": [INPUT_DIM + 1, N_UNIT],
        "rg1_k0": [L, N_UNIT], "rg1_k1": [L, N_UNIT], "rg1_kx": [INPUT_DIM + 1, N_UNIT],
        "ns1_k0": [L, N_UNIT], "ns1_k1": [L, N_UNIT], "ns1_kx": [INPUT_DIM + 1, N_UNIT],
        "ode1_w": [L, N_UNIT], "ode1_b": [1, N_UNIT],
        "ode2_k0": [128, L], "ode2_k1": [128, L], "ode2_b": [1, L],
        "ug2_k0": [128, L], "ug2_k1": [128, L], "ug2_b": [1, L],
        "rg2_k0": [128, L], "rg2_k1": [128, L], "rg2_b": [1, L],
        "ns2_k0": [128, 2 * L], "ns2_k1": [128, 2 * L], "ns2_b": [1, 2 * L],
        "tz1_k0": [L, N_UNIT], "tz1_k1": [L, N_UNIT], "tz1_b": [1, N_UNIT],
        "tz2_k0": [128, 2 * L], "tz2_k1": [128, 2 * L], "tz2_b": [1, 2 * L],
    }
    d_w = {k: nc.dram_tensor(k, v, F32, kind="ExternalInput") for k, v in wspec.items()}

    d_om = nc.dram_tensor("out_m", [L, B], F32, kind="ExternalOutput")
    d_os = nc.dram_tensor("out_s", [L, B], F32, kind="ExternalOutput")

    with tile.TileContext(nc) as tc:
        with (
            tc.tile_pool(name="const", bufs=1) as cp,
            tc.tile_pool(name="work", bufs=2) as wp,
            tc.tile_pool(name="ps", bufs=1, space="PSUM") as pp,
        ):
            # ---- resident constants / weights ----
            w = {}
            for k, shp in wspec.items():
                w[k] = cp.tile(shp, F32, tag=k, name=k)
                nc.sync.dma_start(w[k][:], d_w[k][:])
            dt_sb = cp.tile([128, n_tp], F32, tag="dt_sb", name="dt_sb")
            nc.sync.dma_start(dt_sb[:], d_dt[:])
            ones_row = cp.tile([1, B], F32, tag="ones_row", name="ones_row")
            nc.vector.memset(ones_row[:], 1.0)
            # mask-channel selector: zeros over value rows, ones over mask rows
            msel = cp.tile([INPUT_DIM, 128], F32, tag="msel", name="msel")
            nc.vector.memset(msel[:HALF, :], 0.0)
            nc.vector.memset(msel[HALF:, :], 1.0)

            xbufs = []
            for j in range(3):
                xb = cp.tile([INPUT_DIM + 1, B], F32, tag=f"xb{j}", name=f"xb{j}")
                nc.vector.memset(xb[INPUT_DIM:, :], 1.0)
                xbufs.append(xb)

            ym = [cp.tile([L, B], F32, tag=f"ym{i}", name=f"ym{i}") for i in range(2)]
            ys = [cp.tile([L, B], F32, tag=f"ys{i}", name=f"ys{i}") for i in range(2)]
            nc.vector.memset(ym[0][:], 0.0)
            nc.vector.memset(ys[0][:], 0.0)

            mm = nc.tensor.matmul

            # fp32 matmuls are self-loading and their S3_LW struct fits only
            # ONE sync wait.  Warm the PE's vector clock past every weight
            # DMA with K=1 dummy matmuls so steady-state matmuls only ever
            # wait on their rhs producer (or the psum WAR, via openers).
            scr = pp.tile([1, 16], F32, tag="scr", name="scr")
            for k in wspec:
                mm(scr[0:1, 0:1], w[k][0:1, 0:1], w[k][0:1, 1:2],
                   start=True, stop=True)
            # DVE reads dt_sb (DMA-produced): absorb that first-touch wait too
            warm_dv = cp.tile([1, 4], F32, tag="warm_dv", name="warm_dv")
            nc.vector.tensor_copy(warm_dv[0:1, 0:1], dt_sb[0:1, 0:1])

            # ---- the recurrence ----
            for t in range(n_tp):
                cur, nxt = t % 2, (t + 1) % 2
                xb = xbufs[t % 3]
                nc.sync.dma_start(xb[:INPUT_DIM, :], d_x[t])
                # absorb the x-DMA and ys-producer waits into K=1 dummies so
                # every real matmul below carries at most one sync wait
                mm(scr[0:1, 0:1], xb[0:1, 0:1], xb[0:1, 1:2], start=True, stop=True)
                mm(scr[0:1, 1:2], ys[cur][0:1, 0:1], ys[cur][0:1, 1:2],
                   start=True, stop=True)

                # ODE hidden: tanh(ode_w1^T @ Ym + b1)   -> [2x128u, B]
                psB = pp.tile([128, 2 * B], F32, tag="psB", name="psB")
                for m in range(2):
                    sl = psB[:, m * B:(m + 1) * B]
                    ms = slice(m * 128, (m + 1) * 128)
                    mm(sl, w["ode1_b"][:, ms], ones_row[:], start=True, stop=False)
                    mm(sl, w["ode1_w"][:, ms], ym[cur][:], start=False, stop=True)
                h_ode = wp.tile([128, 2 * B], F32, tag="h_ode", name="h_ode")
                nc.scalar.activation(h_ode[:], psB[:], AF.Tanh)

                # ODE out (+bias) and mask colsum broadcast share one bank
                psF = pp.tile([128, 2 * B], F32, tag="psF", name="psF")
                mm(psF[:, 0:B], w["ode2_b"][:], ones_row[:], start=True, stop=False)
                mm(psF[:, 0:B], w["ode2_k0"][:], h_ode[:, 0:B], start=False, stop=False)
                mm(psF[:, 0:B], w["ode2_k1"][:], h_ode[:, B:], start=False, stop=True)

                # Yode = Ym + dt * ode_out
                yode = wp.tile([L, B], F32, tag="yode", name="yode")
                nc.vector.scalar_tensor_tensor(
                    yode[:], psF[:, 0:B], dt_sb[:, t:t + 1], ym[cur][:],
                    op0=OP.mult, op1=OP.add)

                # update+reset gate layer 1 (4 m-halves in one 2-bank tile)
                psA = pp.tile([128, 4 * B], F32, tag="psA", name="psA")
                for gi, net in enumerate(("ug1", "rg1")):
                    for m in range(2):
                        sl = psA[:, (2 * gi + m) * B:(2 * gi + m + 1) * B]
                        ms = slice(m * 128, (m + 1) * 128)
                        mm(sl, w[net + "_k1"][:, ms], ys[cur][:], start=True, stop=False)
                        mm(sl, w[net + "_kx"][:, ms], xb[:], start=False, stop=False)
                        mm(sl, w[net + "_k0"][:, ms], yode[:], start=False, stop=True)
                # mask colsum broadcast: emitted after the kx matmuls so the
                # x-DMA wait is already in the PE's clock
                mm(psF[:, B:], msel[:], xb[:INPUT_DIM, :], start=True, stop=True)
                h_g1 = wp.tile([128, 4 * B], F32, tag="h_g1", name="h_g1")
                nc.scalar.activation(h_g1[:], psA[:], AF.Tanh)

                # gate layer 2: U | R pre-acts -> tanh(z/2)
                psD = pp.tile([128, 2 * B], F32, tag="psD", name="psD")
                for gi, net in enumerate(("ug2", "rg2")):
                    sl = psD[:, gi * B:(gi + 1) * B]
                    hbase = 2 * gi * B
                    mm(sl, w[net + "_k0"][:], h_g1[:, hbase:hbase + B], start=True, stop=False)
                    mm(sl, w[net + "_b"][:], ones_row[:], start=False, stop=False)
                    mm(sl, w[net + "_k1"][:], h_g1[:, hbase + B:hbase + 2 * B], start=False, stop=True)
                t_ur = wp.tile([128, 2 * B], F32, tag="t_ur", name="t_ur")
                nc.scalar.activation(t_ur[:], psD[:], AF.Tanh, scale=0.5)

                # reset-gated state (carries factor 2; ns1 k0/k1 pre-scaled 0.5)
                am2 = wp.tile([L, B], F32, tag="am2", name="am2")
                nc.vector.scalar_tensor_tensor(
                    am2[:], t_ur[:, B:], 1.0, yode[:], op0=OP.add, op1=OP.mult)
                as2 = wp.tile([L, B], F32, tag="as2", name="as2")
                nc.vector.scalar_tensor_tensor(
                    as2[:], t_ur[:, B:], 1.0, ys[cur][:], op0=OP.add, op1=OP.mult)

                # new-state layer 1
                psC = pp.tile([128, 2 * B], F32, tag="psC", name="psC")
                for m in range(2):
                    sl = psC[:, m * B:(m + 1) * B]
                    ms = slice(m * 128, (m + 1) * 128)
                    mm(sl, w["ns1_kx"][:, ms], xb[:], start=True, stop=False)
                    mm(sl, w["ns1_k0"][:, ms], am2[:], start=False, stop=False)
                    mm(sl, w["ns1_k1"][:, ms], as2[:], start=False, stop=True)
                h_ns = wp.tile([128, 2 * B], F32, tag="h_ns", name="h_ns")
                nc.scalar.activation(h_ns[:], psC[:], AF.Tanh)

                # new-state layer 2: NM | NS pre-acts
                psE = pp.tile([128, 2 * B], F32, tag="psE", name="psE")
                for m in range(2):
                    sl = psE[:, m * B:(m + 1) * B]
                    ms = slice(m * 128, (m + 1) * 128)
                    mm(sl, w["ns2_b"][:, ms], ones_row[:], start=True, stop=False)
                    mm(sl, w["ns2_k0"][:, ms], h_ns[:, 0:B], start=False, stop=False)
                    mm(sl, w["ns2_k1"][:, ms], h_ns[:, B:], start=False, stop=True)

                # G = 0.5*m*(1 - T_u)
                t1 = wp.tile([L, B], F32, tag="t1", name="t1")
                nc.vector.tensor_scalar(t1[:], t_ur[:, 0:B], -0.5, 0.5,
                                        op0=OP.mult, op1=OP.add)
                g = wp.tile([L, B], F32, tag="g", name="g")
                nc.vector.scalar_tensor_tensor(
                    g[:], psF[:, B:], 0.0, t1[:], op0=OP.is_gt, op1=OP.mult)

                # mean channel: Ym' = Yode + G*(NM - Yode)
                dm = wp.tile([L, B], F32, tag="dm", name="dm")
                nc.vector.tensor_tensor(dm[:], psE[:, 0:B], yode[:], op=OP.subtract)
                pm = wp.tile([L, B], F32, tag="pm", name="pm")
                nc.vector.tensor_tensor(pm[:], g[:], dm[:], op=OP.mult)
                nc.vector.tensor_tensor(ym[nxt][:], yode[:], pm[:], op=OP.add)

                # std channel: softplus via Newton (exp-only), then gate
                e_t = wp.tile([L, B], F32, tag="e_t", name="e_t")
                nc.scalar.activation(e_t[:], psE[:, B:], AF.Exp)
                xa = wp.tile([L, B], F32, tag="xa", name="xa")
                nc.scalar.activation(xa[:], psE[:, B:], AF.Abs)
                wx = wp.tile([L, B], F32, tag="wx", name="wx")
                nc.scalar.activation(wx[:], xa[:], AF.Exp, scale=-1.0)
                rl = wp.tile([L, B], F32, tag="rl", name="rl")
                nc.scalar.activation(rl[:], psE[:, B:], AF.Relu)
                a_t = wp.tile([L, B], F32, tag="a_t", name="a_t")
                nc.gpsimd.tensor_scalar(a_t[:], e_t[:], 1.0, None, op0=OP.add)
                y0a = wp.tile([L, B], F32, tag="y0a", name="y0a")
                nc.gpsimd.tensor_scalar(y0a[:], wx[:], LN2, None, op0=OP.mult)
                y_sp = wp.tile([L, B], F32, tag="ysp0", name="ysp0")
                nc.gpsimd.tensor_tensor(y_sp[:], y0a[:], rl[:], op=OP.add)
                for it in range(sp_iters):
                    u_t = wp.tile([L, B], F32, tag=f"usp{it}", name=f"usp{it}")
                    nc.scalar.activation(u_t[:], y_sp[:], AF.Exp, scale=-1.0)
                    tt = wp.tile([L, B], F32, tag=f"tsp{it}", name=f"tsp{it}")
                    nc.vector.tensor_tensor(tt[:], a_t[:], u_t[:], op=OP.mult)
                    y_new = wp.tile([L, B], F32, tag=f"ysp{it + 1}", name=f"ysp{it + 1}")
                    nc.vector.scalar_tensor_tensor(
                        y_new[:], tt[:], -1.0, y_sp[:], op0=OP.add, op1=OP.add)
                    y_sp = y_new
                dsa = wp.tile([L, B], F32, tag="dsa", name="dsa")
                nc.gpsimd.tensor_scalar(dsa[:], y_sp[:], 1e-6, None, op0=OP.add)
                ds = wp.tile([L, B], F32, tag="ds", name="ds")
                nc.gpsimd.tensor_tensor(ds[:], dsa[:], ys[cur][:], op=OP.subtract)
                ps_ = wp.tile([L, B], F32, tag="ps_", name="ps_")
                nc.gpsimd.tensor_tensor(ps_[:], g[:], ds[:], op=OP.mult)
                nc.gpsimd.tensor_tensor(ys[nxt][:], ys[cur][:], ps_[:], op=OP.add)

            # ---- final transform ----
            fin = n_tp % 2
            psB = pp.tile([128, 2 * B], F32, tag="psB", name="psB")
            for m in range(2):
                sl = psB[:, m * B:(m + 1) * B]
                ms = slice(m * 128, (m + 1) * 128)
                mm(sl, w["tz1_b"][:, ms], ones_row[:], start=True, stop=False)
                mm(sl, w["tz1_k0"][:, ms], ym[fin][:], start=False, stop=False)
                mm(sl, w["tz1_k1"][:, ms], ys[fin][:], start=False, stop=True)
            h_tz = wp.tile([128, 2 * B], F32, tag="h_ode", name="h_ode")
            nc.scalar.activation(h_tz[:], psB[:], AF.Tanh)
            psE = pp.tile([128, 2 * B], F32, tag="psE", name="psE")
            for m in range(2):
                sl = psE[:, m * B:(m + 1) * B]
                ms = slice(m * 128, (m + 1) * 128)
                mm(sl, w["tz2_b"][:, ms], ones_row[:], start=True, stop=False)
                mm(sl, w["tz2_k0"][:, ms], h_tz[:, 0:B], start=False, stop=False)
                mm(sl, w["tz2_k1"][:, ms], h_tz[:, B:], start=False, stop=True)
            o_m = wp.tile([L, B], F32, tag="o_m", name="o_m")
            nc.scalar.activation(o_m[:], psE[:, 0:B], AF.Copy)
            o_s = wp.tile([L, B], F32, tag="o_s", name="o_s")
            nc.scalar.activation(o_s[:], psE[:, B:], AF.Abs)
            nc.sync.dma_start(d_om[:], o_m[:])
            nc.sync.dma_start(d_os[:], o_s[:])

    nc.compile()
    return nc


# --------------------------------------------------------------------------
# host-side packing
# --------------------------------------------------------------------------
def _prep_in_maps(inputs, n_tp):
    F = np.float32
    d = {k: np.ascontiguousarray(np.asarray(v, F)) for k, v in inputs.items()}
    obs = d["obs_tps"][:n_tp]
    data = d["data"][:, :n_tp]

    dd = (obs[:-1] - obs[1:])[::-1]
    dts = np.concatenate([np.full((1,), -0.01, F), dd])
    dt_b = np.ascontiguousarray(np.broadcast_to(dts[None, :], (128, n_tp)))

    # [t, c, subj], reversed in time
    x_rev = np.ascontiguousarray(data.transpose(1, 2, 0)[::-1])

    ns_w1s = d["ns_w1"].copy()
    ns_w1s[:2 * L] *= F(0.5)

    def kx(w1, b1):
        return np.ascontiguousarray(np.vstack([w1[2 * L:], b1[None, :]]))

    shared = {
        "dt_b": dt_b,
        "ug1_k0": d["ug_w1"][:L], "ug1_k1": d["ug_w1"][L:2 * L],
        "ug1_kx": kx(d["ug_w1"], d["ug_b1"]),
        "rg1_k0": d["rg_w1"][:L], "rg1_k1": d["rg_w1"][L:2 * L],
        "rg1_kx": kx(d["rg_w1"], d["rg_b1"]),
        "ns1_k0": ns_w1s[:L], "ns1_k1": ns_w1s[L:2 * L],
        "ns1_kx": kx(d["ns_w1"], d["ns_b1"]),
        "ode1_w": d["ode_w1"], "ode1_b": d["ode_b1"][None, :],
        "ode2_k0": d["ode_w2"][:128], "ode2_k1": d["ode_w2"][128:],
        "ode2_b": d["ode_b2"][None, :],
        "ug2_k0": d["ug_w2"][:128], "ug2_k1": d["ug_w2"][128:],
        "ug2_b": d["ug_b2"][None, :],
        "rg2_k0": d["rg_w2"][:128], "rg2_k1": d["rg_w2"][128:],
        "rg2_b": d["rg_b2"][None, :],
        "ns2_k0": d["ns_w2"][:128], "ns2_k1": d["ns_w2"][128:],
        "ns2_b": d["ns_b2"][None, :],
        "tz1_k0": d["tz_w1"][:L], "tz1_k1": d["tz_w1"][L:],
        "tz1_b": d["tz_b1"][None, :],
        "tz2_k0": d["tz_w2"][:128], "tz2_k1": d["tz_w2"][128:],
        "tz2_b": d["tz_b2"][None, :],
    }
    shared = {k: np.ascontiguousarray(v) for k, v in shared.items()}

    in_maps = []
    for c in range(N_CORES):
        m = dict(shared)
        m["x_rev"] = np.ascontiguousarray(x_rev[:, :, c * B:(c + 1) * B])
        in_maps.append(m)
    return in_maps


def kernel(**inputs):
    from concourse.bass_utils import run_bass_kernel_spmd

    key = (N_TP, SP_ITERS)
    if key not in _CACHE:
        _CACHE[key] = _build(*key)
    nc = _CACHE[key]

    in_maps = _prep_in_maps(inputs, N_TP)
    res = run_bass_kernel_spmd(nc, in_maps, list(range(N_CORES)))
    outs = res.results

    mean = np.empty((1, N_SUBJ, L), np.float32)
    std = np.empty((1, N_SUBJ, L), np.float32)
    for c in range(N_CORES):
        mean[0, c * B:(c + 1) * B] = outs[c]["out_m"].T
        std[0, c * B:(c + 1) * B] = outs[c]["out_s"].T
    return mean, std


# revision 21
# speedup vs baseline: 1.7547x; 1.7547x over previous
"""Trainium2 Bass kernel for an ODE-RNN encoder (z0 posterior).

Model: 128-step reversed-time GRU-like recurrence with an Euler ODE step on
the mean channel, then a final transform producing (mean_z0, std_z0).

Strategy: data-parallel over the subject (batch) dim across 8 NeuronCores,
weights replicated.  Everything runs on-chip in a transposed layout
([feature, batch], batch=256 on the free dim).  Key points:
- matmul operands are bf16 (fp32 PSUM accumulate): fp32 matmuls on TRN2
  lower to TWO half-speed PE passes, bf16 is single-pass with fast weight
  load.  State tensors stay fp32; small bf16 copies feed the PE.
  (host-validated: ~5e-3 max rel error vs the fp32 reference)
- layer-1 biases ride a ones-row appended to the streamed x tile; the
  remaining biases use ACT bias vectors, a fused scalar_tensor_tensor, or
  a cheap K=1 ones-row matmul;
- sigmoid(z) = 0.5 + 0.5*tanh(z/2) keeps every transcendental in the
  resident `exp_and_others` ACT table set (no per-step table switches);
- softplus(x) = log1p(exp(x)) via Newton on y: e^y = 1 + e^x, seeded with
  y0 = relu(x) + ln2*exp(-|x|) (1 iteration: ~1.2e-3, under the bf16 floor);
- the update gate, the observation mask (broadcast via an all-ones K=64
  selector matmul) and the GRU convex combination fold into one factor
  G = 0.5*m*(1-tanh(zU/2)):  new = old + G*(cand - old);
- TRN2 allows ONE sync wait per instruction; Bacc legalizes the rest, but
  K=1 dummy matmuls + accumulation-group "openers" keep the PE free of
  multi-wait event-semaphore preambles in the steady state.
"""
import sys
import numpy as np
import ml_dtypes

for _p in ("/opt/trn_rl_repo", "/root/.axon_site/_ro/trn_rl_repo"):
    if _p not in sys.path:
        sys.path.append(_p)

N_SUBJ, N_TP, INPUT_DIM, LATENT, N_UNIT = 2048, 128, 64, 128, 256
HALF = INPUT_DIM // 2
N_CORES = 8
B = N_SUBJ // N_CORES          # 256 subjects per core (free dim)
L = LATENT
SP_ITERS = 1                   # softplus Newton refinements
LN2 = float(np.log(2.0))
BF = ml_dtypes.bfloat16

_CACHE = {}


# --------------------------------------------------------------------------
# Bass program
# --------------------------------------------------------------------------
def _build(n_tp, sp_iters):
    import concourse.mybir as mybir
    from concourse import bacc, tile

    F32 = mybir.dt.float32
    B16 = mybir.dt.bfloat16
    AF = mybir.ActivationFunctionType
    OP = mybir.AluOpType

    # Bacc (not plain Bass): its compile() legalizes the TRN2 one-sync-wait-
    # per-instruction limit (event-semaphore splitting, matmul-wait moves).
    nc = bacc.Bacc(None)

    # ---- DRAM I/O ----
    d_x = nc.dram_tensor("x_rev", [n_tp, INPUT_DIM, B], B16, kind="ExternalInput")
    d_dt = nc.dram_tensor("dt_b", [128, n_tp], F32, kind="ExternalInput")

    bspec = {  # bf16 weights (matmul operands)
        "ug1_k0": [L, N_UNIT], "ug1_k1": [L, N_UNIT], "ug1_kx": [INPUT_DIM + 1, N_UNIT],
        "rg1_k0": [L, N_UNIT], "rg1_k1": [L, N_UNIT], "rg1_kx": [INPUT_DIM + 1, N_UNIT],
        "ns1_k0": [L, N_UNIT], "ns1_k1": [L, N_UNIT], "ns1_kx": [INPUT_DIM + 1, N_UNIT],
        "ode1_w": [L, N_UNIT], "ode1_b": [1, N_UNIT],
        "ode2_k0": [128, L], "ode2_k1": [128, L], "ode2_b": [1, L],
        "ug2_k0": [128, L], "ug2_k1": [128, L], "ug2_b": [1, L],
        "rg2_k0": [128, L], "rg2_k1": [128, L], "rg2_b": [1, L],
        "ns2_k0": [128, 2 * L], "ns2_k1": [128, 2 * L],
        "tz1_k0": [L, N_UNIT], "tz1_k1": [L, N_UNIT], "tz1_b": [1, N_UNIT],
        "tz2_k0": [128, 2 * L], "tz2_k1": [128, 2 * L],
    }
    fspec = {  # fp32 per-partition bias columns (ACT bias / STT scalar APs)
        "ns2_bm": [128, 1], "ns2_bs": [128, 1],
        "tz2_bm": [128, 1], "tz2_bs": [128, 1],
    }
    d_w = {k: nc.dram_tensor(k, v, B16, kind="ExternalInput") for k, v in bspec.items()}
    d_w.update({k: nc.dram_tensor(k, v, F32, kind="ExternalInput")
                for k, v in fspec.items()})

    d_om = nc.dram_tensor("out_m", [L, B], F32, kind="ExternalOutput")
    d_os = nc.dram_tensor("out_s", [L, B], F32, kind="ExternalOutput")

    with tile.TileContext(nc) as tc:
        with (
            tc.tile_pool(name="const", bufs=1) as cp,
            tc.tile_pool(name="work", bufs=2) as wp,
            tc.tile_pool(name="ps", bufs=1, space="PSUM") as pp,
        ):
            # ---- resident constants / weights ----
            w = {}
            for k, shp in bspec.items():
                w[k] = cp.tile(shp, B16, tag=k, name=k)
                nc.sync.dma_start(w[k][:], d_w[k][:])
            for k, shp in fspec.items():
                w[k] = cp.tile(shp, F32, tag=k, name=k)
                nc.sync.dma_start(w[k][:], d_w[k][:])
            dt_sb = cp.tile([128, n_tp], F32, tag="dt_sb", name="dt_sb")
            nc.sync.dma_start(dt_sb[:], d_dt[:])
            ones_row = cp.tile([1, B], B16, tag="ones_row", name="ones_row")
            nc.vector.memset(ones_row[:], 1.0)
            # mask-channel selector: zeros over value rows, ones over mask rows
            msel = cp.tile([INPUT_DIM, 128], B16, tag="msel", name="msel")
            nc.vector.memset(msel[:HALF, :], 0.0)
            nc.vector.memset(msel[HALF:, :], 1.0)

            xbufs = []
            for j in range(3):
                xb = cp.tile([INPUT_DIM + 1, B], B16, tag=f"xb{j}", name=f"xb{j}")
                nc.vector.memset(xb[INPUT_DIM:, :], 1.0)
                xbufs.append(xb)

            ym = [cp.tile([L, B], F32, tag=f"ym{i}", name=f"ym{i}") for i in range(2)]
            ys = [cp.tile([L, B], F32, tag=f"ys{i}", name=f"ys{i}") for i in range(2)]
            ymb = cp.tile([L, B], B16, tag="ymb", name="ymb")
            ysb = cp.tile([L, B], B16, tag="ysb", name="ysb")
            nc.vector.memset(ym[0][:], 0.0)
            nc.vector.memset(ys[0][:], 0.0)
            nc.vector.memset(ymb[:], 0.0)
            nc.vector.memset(ysb[:], 0.0)

            mm = nc.tensor.matmul

            # Warm the PE's clock past every weight DMA with K=1 dummy
            # matmuls so steady-state matmuls only wait on one producer.
            scr = pp.tile([1, 16], F32, tag="scr", name="scr")
            for k in bspec:
                mm(scr[0:1, 0:1], w[k][0:1, 0:1], w[k][0:1, 1:2],
                   start=True, stop=True)
            # DVE reads dt_sb / bias columns (DMA-produced): warm those too
            warm_dv = cp.tile([1, 8], F32, tag="warm_dv", name="warm_dv")
            nc.vector.tensor_copy(warm_dv[0:1, 0:1], dt_sb[0:1, 0:1])
            for j, k in enumerate(fspec):
                nc.vector.tensor_copy(warm_dv[0:1, j + 1:j + 2], w[k][0:1, 0:1])

            # ---- the recurrence ----
            for t in range(n_tp):
                cur, nxt = t % 2, (t + 1) % 2
                xb = xbufs[t % 3]
                nc.sync.dma_start(xb[:INPUT_DIM, :], d_x[t])
                # absorb the x-DMA / state-producer waits into K=1 dummies
                mm(scr[0:1, 0:1], xb[0:1, 0:1], xb[0:1, 1:2], start=True, stop=True)
                mm(scr[0:1, 1:2], ysb[0:1, 0:1], ysb[0:1, 1:2], start=True, stop=True)

                # ODE hidden: tanh(ode_w1^T @ Ym + b1)   -> [2x128u, B]
                psB = pp.tile([128, 2 * B], F32, tag="psB", name="psB")
                for m in range(2):
                    sl = psB[:, m * B:(m + 1) * B]
                    ms = slice(m * 128, (m + 1) * 128)
                    mm(sl, w["ode1_b"][:, ms], ones_row[:], start=True, stop=False)
                    mm(sl, w["ode1_w"][:, ms], ymb[:], start=False, stop=True)
                h_ode = wp.tile([128, 2 * B], B16, tag="h_ode", name="h_ode")
                nc.scalar.activation(h_ode[:], psB[:], AF.Tanh)

                # ODE out (+bias) and mask colsum broadcast share one bank
                psF = pp.tile([128, 2 * B], F32, tag="psF", name="psF")
                mm(psF[:, 0:B], w["ode2_b"][:], ones_row[:], start=True, stop=False)
                mm(psF[:, 0:B], w["ode2_k0"][:], h_ode[:, 0:B], start=False, stop=False)
                mm(psF[:, 0:B], w["ode2_k1"][:], h_ode[:, B:], start=False, stop=True)

                # Yode = Ym + dt * ode_out   (fp32), plus a bf16 copy for PE
                yode = wp.tile([L, B], F32, tag="yode", name="yode")
                nc.vector.scalar_tensor_tensor(
                    yode[:], psF[:, 0:B], dt_sb[:, t:t + 1], ym[cur][:],
                    op0=OP.mult, op1=OP.add)
                yodeb = wp.tile([L, B], B16, tag="yodeb", name="yodeb")
                nc.gpsimd.tensor_copy(yodeb[:], yode[:])

                # update+reset gate layer 1 (4 m-halves in one 2-bank tile)
                psA = pp.tile([128, 4 * B], F32, tag="psA", name="psA")
                for gi, net in enumerate(("ug1", "rg1")):
                    for m in range(2):
                        sl = psA[:, (2 * gi + m) * B:(2 * gi + m + 1) * B]
                        ms = slice(m * 128, (m + 1) * 128)
                        mm(sl, w[net + "_k1"][:, ms], ysb[:], start=True, stop=False)
                        mm(sl, w[net + "_kx"][:, ms], xb[:], start=False, stop=False)
                        mm(sl, w[net + "_k0"][:, ms], yodeb[:], start=False, stop=True)
                # mask colsum broadcast, after the kx matmuls (x-DMA wait seen)
                mm(psF[:, B:], msel[:], xb[:INPUT_DIM, :], start=True, stop=True)
                h_g1 = wp.tile([128, 4 * B], B16, tag="h_g1", name="h_g1")
                nc.scalar.activation(h_g1[:], psA[:], AF.Tanh)

                # gate layer 2: U | R pre-acts -> tanh(z/2)
                psD = pp.tile([128, 2 * B], F32, tag="psD", name="psD")
                for gi, net in enumerate(("ug2", "rg2")):
                    sl = psD[:, gi * B:(gi + 1) * B]
                    hbase = 2 * gi * B
                    mm(sl, w[net + "_k0"][:], h_g1[:, hbase:hbase + B], start=True, stop=False)
                    mm(sl, w[net + "_b"][:], ones_row[:], start=False, stop=False)
                    mm(sl, w[net + "_k1"][:], h_g1[:, hbase + B:hbase + 2 * B], start=False, stop=True)
                t_ur = wp.tile([128, 2 * B], B16, tag="t_ur", name="t_ur")
                nc.scalar.activation(t_ur[:], psD[:], AF.Tanh, scale=0.5)

                # reset-gated state (carries factor 2; ns1 k0/k1 pre-scaled 0.5)
                am2 = wp.tile([L, B], B16, tag="am2", name="am2")
                nc.vector.scalar_tensor_tensor(
                    am2[:], t_ur[:, B:], 1.0, yode[:], op0=OP.add, op1=OP.mult)
                as2 = wp.tile([L, B], B16, tag="as2", name="as2")
                nc.vector.scalar_tensor_tensor(
                    as2[:], t_ur[:, B:], 1.0, ys[cur][:], op0=OP.add, op1=OP.mult)

                # new-state layer 1
                psC = pp.tile([128, 2 * B], F32, tag="psC", name="psC")
                for m in range(2):
                    sl = psC[:, m * B:(m + 1) * B]
                    ms = slice(m * 128, (m + 1) * 128)
                    mm(sl, w["ns1_kx"][:, ms], xb[:], start=True, stop=False)
                    mm(sl, w["ns1_k0"][:, ms], am2[:], start=False, stop=False)
                    mm(sl, w["ns1_k1"][:, ms], as2[:], start=False, stop=True)
                h_ns = wp.tile([128, 2 * B], B16, tag="h_ns", name="h_ns")
                nc.scalar.activation(h_ns[:], psC[:], AF.Tanh)

                # new-state layer 2: NM | NS pre-acts (biases via APs below)
                psE = pp.tile([128, 2 * B], F32, tag="psE", name="psE")
                for m in range(2):
                    sl = psE[:, m * B:(m + 1) * B]
                    ms = slice(m * 128, (m + 1) * 128)
                    mm(sl, w["ns2_k0"][:, ms], h_ns[:, 0:B], start=True, stop=False)
                    mm(sl, w["ns2_k1"][:, ms], h_ns[:, B:], start=False, stop=True)

                # G = 0.5*m*(1 - T_u)
                t1 = wp.tile([L, B], F32, tag="t1", name="t1")
                nc.vector.tensor_scalar(t1[:], t_ur[:, 0:B], -0.5, 0.5,
                                        op0=OP.mult, op1=OP.add)
                g = wp.tile([L, B], F32, tag="g", name="g")
                nc.vector.scalar_tensor_tensor(
                    g[:], psF[:, B:], 0.0, t1[:], op0=OP.is_gt, op1=OP.mult)

                # mean channel: Ym' = Yode + G*((NM+bm) - Yode)
                dm = wp.tile([L, B], F32, tag="dm", name="dm")
                nc.vector.scalar_tensor_tensor(
                    dm[:], psE[:, 0:B], w["ns2_bm"][:, 0:1], yode[:],
                    op0=OP.add, op1=OP.subtract)
                pm = wp.tile([L, B], F32, tag="pm", name="pm")
                nc.vector.tensor_tensor(pm[:], g[:], dm[:], op=OP.mult)
                nc.vector.tensor_tensor(ym[nxt][:], yode[:], pm[:], op=OP.add)
                nc.gpsimd.tensor_copy(ymb[:], ym[nxt][:])

                # std channel: softplus(x)=log1p(e^x) via Newton, then gate
                e_t = wp.tile([L, B], F32, tag="e_t", name="e_t")
                nc.scalar.activation(e_t[:], psE[:, B:], AF.Exp,
                                     bias=w["ns2_bs"][:, 0:1])
                xa = wp.tile([L, B], F32, tag="xa", name="xa")
                nc.scalar.activation(xa[:], psE[:, B:], AF.Abs,
                                     bias=w["ns2_bs"][:, 0:1])
                wx = wp.tile([L, B], F32, tag="wx", name="wx")
                nc.scalar.activation(wx[:], xa[:], AF.Exp, scale=-1.0)
                rl = wp.tile([L, B], F32, tag="rl", name="rl")
                nc.vector.tensor_scalar(rl[:], psE[:, B:], w["ns2_bs"][:, 0:1],
                                        0.0, op0=OP.add, op1=OP.max)
                a_t = wp.tile([L, B], F32, tag="a_t", name="a_t")
                nc.gpsimd.tensor_scalar(a_t[:], e_t[:], 1.0, None, op0=OP.add)
                y_sp = wp.tile([L, B], F32, tag="ysp0", name="ysp0")
                nc.vector.scalar_tensor_tensor(
                    y_sp[:], wx[:], LN2, rl[:], op0=OP.mult, op1=OP.add)
                for it in range(sp_iters):
                    u_t = wp.tile([L, B], F32, tag=f"usp{it}", name=f"usp{it}")
                    nc.scalar.activation(u_t[:], y_sp[:], AF.Exp, scale=-1.0)
                    tt = wp.tile([L, B], F32, tag=f"tsp{it}", name=f"tsp{it}")
                    nc.vector.tensor_tensor(tt[:], a_t[:], u_t[:], op=OP.mult)
                    y_new = wp.tile([L, B], F32, tag=f"ysp{it + 1}", name=f"ysp{it + 1}")
                    nc.vector.scalar_tensor_tensor(
                        y_new[:], tt[:], -1.0, y_sp[:], op0=OP.add, op1=OP.add)
                    y_sp = y_new
                ds = wp.tile([L, B], F32, tag="ds", name="ds")
                nc.vector.scalar_tensor_tensor(
                    ds[:], y_sp[:], 1e-6, ys[cur][:], op0=OP.add, op1=OP.subtract)
                ps_ = wp.tile([L, B], F32, tag="ps_", name="ps_")
                nc.vector.tensor_tensor(ps_[:], g[:], ds[:], op=OP.mult)
                nc.vector.tensor_tensor(ys[nxt][:], ys[cur][:], ps_[:], op=OP.add)
                nc.gpsimd.tensor_copy(ysb[:], ys[nxt][:])

            # ---- final transform ----
            fin = n_tp % 2
            psB = pp.tile([128, 2 * B], F32, tag="psB", name="psB")
            for m in range(2):
                sl = psB[:, m * B:(m + 1) * B]
                ms = slice(m * 128, (m + 1) * 128)
                mm(sl, w["tz1_b"][:, ms], ones_row[:], start=True, stop=False)
                mm(sl, w["tz1_k0"][:, ms], ymb[:], start=False, stop=False)
                mm(sl, w["tz1_k1"][:, ms], ysb[:], start=False, stop=True)
            h_tz = wp.tile([128, 2 * B], B16, tag="h_ode", name="h_tz")
            nc.scalar.activation(h_tz[:], psB[:], AF.Tanh)
            psE = pp.tile([128, 2 * B], F32, tag="psE", name="psE2")
            for m in range(2):
                sl = psE[:, m * B:(m + 1) * B]
                ms = slice(m * 128, (m + 1) * 128)
                mm(sl, w["tz2_k0"][:, ms], h_tz[:, 0:B], start=True, stop=False)
                mm(sl, w["tz2_k1"][:, ms], h_tz[:, B:], start=False, stop=True)
            o_m = wp.tile([L, B], F32, tag="o_m", name="o_m")
            nc.scalar.activation(o_m[:], psE[:, 0:B], AF.Identity,
                                 bias=w["tz2_bm"][:, 0:1])
            o_s = wp.tile([L, B], F32, tag="o_s", name="o_s")
            nc.scalar.activation(o_s[:], psE[:, B:], AF.Abs,
                                 bias=w["tz2_bs"][:, 0:1])
            nc.sync.dma_start(d_om[:], o_m[:])
            nc.sync.dma_start(d_os[:], o_s[:])

    nc.compile()
    return nc


# --------------------------------------------------------------------------
# host-side packing
# --------------------------------------------------------------------------
def _prep_in_maps(inputs, n_tp):
    F = np.float32
    d = {k: np.ascontiguousarray(np.asarray(v, F)) for k, v in inputs.items()}
    obs = d["obs_tps"][:n_tp]
    data = d["data"][:, :n_tp]

    dd = (obs[:-1] - obs[1:])[::-1]
    dts = np.concatenate([np.full((1,), -0.01, F), dd])
    dt_b = np.ascontiguousarray(np.broadcast_to(dts[None, :], (128, n_tp)))

    # [t, c, subj], reversed in time, bf16
    x_rev = np.ascontiguousarray(data.transpose(1, 2, 0)[::-1]).astype(BF)

    ns_w1s = d["ns_w1"].copy()
    ns_w1s[:2 * L] *= F(0.5)

    def kx(w1, b1):
        return np.vstack([w1[2 * L:], b1[None, :]])

    bf = {
        "ug1_k0": d["ug_w1"][:L], "ug1_k1": d["ug_w1"][L:2 * L],
        "ug1_kx": kx(d["ug_w1"], d["ug_b1"]),
        "rg1_k0": d["rg_w1"][:L], "rg1_k1": d["rg_w1"][L:2 * L],
        "rg1_kx": kx(d["rg_w1"], d["rg_b1"]),
        "ns1_k0": ns_w1s[:L], "ns1_k1": ns_w1s[L:2 * L],
        "ns1_kx": kx(d["ns_w1"], d["ns_b1"]),
        "ode1_w": d["ode_w1"], "ode1_b": d["ode_b1"][None, :],
        "ode2_k0": d["ode_w2"][:128], "ode2_k1": d["ode_w2"][128:],
        "ode2_b": d["ode_b2"][None, :],
        "ug2_k0": d["ug_w2"][:128], "ug2_k1": d["ug_w2"][128:],
        "ug2_b": d["ug_b2"][None, :],
        "rg2_k0": d["rg_w2"][:128], "rg2_k1": d["rg_w2"][128:],
        "rg2_b": d["rg_b2"][None, :],
        "ns2_k0": d["ns_w2"][:128], "ns2_k1": d["ns_w2"][128:],
        "tz1_k0": d["tz_w1"][:L], "tz1_k1": d["tz_w1"][L:],
        "tz1_b": d["tz_b1"][None, :],
        "tz2_k0": d["tz_w2"][:128], "tz2_k1": d["tz_w2"][128:],
    }
    shared = {k: np.ascontiguousarray(v.astype(BF)) for k, v in bf.items()}
    shared["dt_b"] = dt_b
    shared["ns2_bm"] = np.ascontiguousarray(d["ns_b2"][:L, None])
    shared["ns2_bs"] = np.ascontiguousarray(d["ns_b2"][L:, None])
    shared["tz2_bm"] = np.ascontiguousarray(d["tz_b2"][:L, None])
    shared["tz2_bs"] = np.ascontiguousarray(d["tz_b2"][L:, None])

    in_maps = []
    for c in range(N_CORES):
        m = dict(shared)
        m["x_rev"] = np.ascontiguousarray(x_rev[:, :, c * B:(c + 1) * B])
        in_maps.append(m)
    return in_maps


def kernel(**inputs):
    from concourse.bass_utils import run_bass_kernel_spmd

    key = (N_TP, SP_ITERS)
    if key not in _CACHE:
        _CACHE[key] = _build(*key)
    nc = _CACHE[key]

    in_maps = _prep_in_maps(inputs, N_TP)
    res = run_bass_kernel_spmd(nc, in_maps, list(range(N_CORES)))
    outs = res.results

    mean = np.empty((1, N_SUBJ, L), np.float32)
    std = np.empty((1, N_SUBJ, L), np.float32)
    for c in range(N_CORES):
        mean[0, c * B:(c + 1) * B] = outs[c]["out_m"].T
        std[0, c * B:(c + 1) * B] = outs[c]["out_s"].T
    return mean, std


# revision 30
# speedup vs baseline: 2.3550x; 1.3421x over previous
"""Trainium2 Bass kernel for an ODE-RNN encoder (z0 posterior).

Model: 128-step reversed-time GRU-like recurrence with an Euler ODE step on
the mean channel, then a final transform producing (mean_z0, std_z0).

Strategy: data-parallel over the subject (batch) dim across 8 NeuronCores,
weights replicated.  Everything runs on-chip in a transposed layout
([feature, batch], batch=256 on the free dim).  Key points:
- matmul operands are bf16 (fp32 PSUM accumulate): fp32 matmuls on TRN2
  lower to TWO half-speed PE passes, bf16 is single-pass with fast weight
  load.  State tensors stay fp32; small bf16 copies feed the PE.
  (host-validated: ~5e-3 max rel error vs the fp32 reference)
- all biases ride either a ones-row appended to the streamed x tile
  (layer-1 nets) or ACT per-partition bias vectors — zero per-step bias
  matmuls;  Yode = Ym + dt*(ode_out + b2) becomes one ACT op with
  per-partition scale=dt and bias=dt*b2 (host-precomputed per step);
- sigmoid(z) = 0.5 + 0.5*tanh(z/2) keeps every transcendental in the
  resident `exp_and_others` ACT table set (no per-step table switches);
- softplus(x) = log1p(exp(x)) via Newton on y: e^y = 1 + e^x, seeded with
  y0 = relu(x) + ln2*exp(-|x|) (1 iteration: ~1.2e-3, under the bf16 floor);
- the update gate, the observation mask (broadcast via an all-ones K=64
  selector matmul) and the GRU convex combination fold into one factor
  G = 0.5*m*(1-tanh(zU/2)):  new = old + G*(cand - old); the (cand - old)
  part is accumulated in PSUM via a negative-identity matmul;
- TRN2 allows ONE sync wait per instruction; Bacc legalizes the rest, but
  K=1 dummy matmuls + accumulation-group ordering keep the PE free of
  multi-wait event-semaphore preambles in the steady state.
"""
import sys
import numpy as np
import ml_dtypes

for _p in ("/opt/trn_rl_repo", "/root/.axon_site/_ro/trn_rl_repo"):
    if _p not in sys.path:
        sys.path.append(_p)

N_SUBJ, N_TP, INPUT_DIM, LATENT, N_UNIT = 2048, 128, 64, 128, 256
HALF = INPUT_DIM // 2
N_CORES = 8
B = N_SUBJ // N_CORES          # 256 subjects per core (free dim)
L = LATENT
SP_ITERS = 1                   # softplus Newton refinements
LN2 = float(np.log(2.0))
BF = ml_dtypes.bfloat16

_CACHE = {}


# --------------------------------------------------------------------------
# Bass program
# --------------------------------------------------------------------------
def _build(n_tp, sp_iters):
    import concourse.mybir as mybir
    from concourse import bacc, tile

    F32 = mybir.dt.float32
    B16 = mybir.dt.bfloat16
    AF = mybir.ActivationFunctionType
    OP = mybir.AluOpType

    # Bacc (not plain Bass): its compile() legalizes the TRN2 one-sync-wait-
    # per-instruction limit (event-semaphore splitting, matmul-wait moves).
    nc = bacc.Bacc(None)

    # ---- DRAM I/O ----
    d_x = nc.dram_tensor("x_rev", [n_tp, INPUT_DIM, B], B16, kind="ExternalInput")

    bspec = {  # bf16 weights (matmul operands)
        "ug1_k0": [L, N_UNIT], "ug1_k1": [L, N_UNIT], "ug1_kx": [INPUT_DIM + 1, N_UNIT],
        "rg1_k0": [L, N_UNIT], "rg1_k1": [L, N_UNIT], "rg1_kx": [INPUT_DIM + 1, N_UNIT],
        "ns1_k0": [L, N_UNIT], "ns1_k1": [L, N_UNIT], "ns1_kx": [INPUT_DIM + 1, N_UNIT],
        "ode1_w": [L, N_UNIT],
        "ode2_k0": [128, L], "ode2_k1": [128, L],
        "ug2_k0": [128, L], "ug2_k1": [128, L],
        "rg2_k0": [128, L], "rg2_k1": [128, L],
        "ns2_k0": [128, 2 * L], "ns2_k1": [128, 2 * L], "ns2_bm16": [1, L],
        "neg_eye": [L, L],
        "tz1_k0": [L, N_UNIT], "tz1_k1": [L, N_UNIT], "tz1_b": [1, N_UNIT],
        "tz2_k0": [128, 2 * L], "tz2_k1": [128, 2 * L],
    }
    fspec = {  # fp32 per-partition columns (ACT bias/scale, STT scalar APs)
        "ode1_bc": [128, 2], "ug2_bc": [128, 1], "rg2_bc": [128, 1],
        "ns2_bm": [128, 1], "ns2_bs": [128, 1], "tz2_bm": [128, 1], "tz2_bs": [128, 1],
        "dt_b": [128, n_tp], "b2dt": [128, n_tp],
    }
    d_w = {k: nc.dram_tensor(k, v, B16, kind="ExternalInput") for k, v in bspec.items()}
    d_w.update({k: nc.dram_tensor(k, v, F32, kind="ExternalInput")
                for k, v in fspec.items()})

    d_om = nc.dram_tensor("out_m", [L, B], F32, kind="ExternalOutput")
    d_os = nc.dram_tensor("out_s", [L, B], F32, kind="ExternalOutput")

    with tile.TileContext(nc) as tc:
        with (
            tc.tile_pool(name="const", bufs=1) as cp,
            tc.tile_pool(name="work", bufs=3) as wp,
            tc.tile_pool(name="ps", bufs=1, space="PSUM") as pp,
        ):
            # ---- resident constants / weights ----
            w = {}
            for k, shp in bspec.items():
                w[k] = cp.tile(shp, B16, tag=k, name=k)
                nc.sync.dma_start(w[k][:], d_w[k][:])
            for k, shp in fspec.items():
                w[k] = cp.tile(shp, F32, tag=k, name=k)
                nc.sync.dma_start(w[k][:], d_w[k][:])
            ones_row = cp.tile([1, B], B16, tag="ones_row", name="ones_row")
            nc.vector.memset(ones_row[:], 1.0)
            # mask-channel selector: zeros over value rows, ones over mask rows
            msel = cp.tile([INPUT_DIM, 128], B16, tag="msel", name="msel")
            nc.vector.memset(msel[:HALF, :], 0.0)
            nc.vector.memset(msel[HALF:, :], 1.0)

            xbufs = []
            for j in range(3):
                xb = cp.tile([INPUT_DIM + 1, B], B16, tag=f"xb{j}", name=f"xb{j}")
                nc.vector.memset(xb[INPUT_DIM:, :], 1.0)
                xbufs.append(xb)

            ym = [cp.tile([L, B], F32, tag=f"ym{i}", name=f"ym{i}") for i in range(2)]
            ys = [cp.tile([L, B], F32, tag=f"ys{i}", name=f"ys{i}") for i in range(2)]
            ymb = cp.tile([L, B], B16, tag="ymb", name="ymb")
            ysb = cp.tile([L, B], B16, tag="ysb", name="ysb")
            nc.vector.memset(ym[0][:], 0.0)
            nc.vector.memset(ys[0][:], 0.0)
            nc.vector.memset(ymb[:], 0.0)
            nc.vector.memset(ysb[:], 0.0)

            mm = nc.tensor.matmul

            # Warm the PE's clock past every weight DMA with K=1 dummy
            # matmuls so steady-state matmuls only wait on one producer.
            scr = pp.tile([1, 16], F32, tag="scr", name="scr")
            for k in bspec:
                mm(scr[0:1, 0:1], w[k][0:1, 0:1], w[k][0:1, 1:2],
                   start=True, stop=True)
            # DVE/ACT read fp32 DMA-produced columns: warm those clocks too
            nf = len(fspec)
            warm_dv = cp.tile([1, 2 * nf], F32, tag="warm_dv", name="warm_dv")
            for j, k in enumerate(fspec):
                nc.vector.tensor_copy(warm_dv[0:1, j:j + 1], w[k][0:1, 0:1])
                nc.scalar.copy(warm_dv[0:1, nf + j:nf + j + 1], w[k][0:1, 0:1])

            # ---- the recurrence ----
            for t in range(n_tp):
                cur, nxt = t % 2, (t + 1) % 2
                xb = xbufs[t % 3]
                nc.sync.dma_start(xb[:INPUT_DIM, :], d_x[t])
                # absorb the x-DMA / state-producer waits into K=1 dummies
                mm(scr[0:1, 0:1], xb[0:1, 0:1], xb[0:1, 1:2], start=True, stop=True)
                mm(scr[0:1, 1:2], ysb[0:1, 0:1], ysb[0:1, 1:2], start=True, stop=True)

                # ODE hidden: tanh(ode_w1^T @ Ym + b1); split per m-half so
                # the ode2 k0 matmul starts as soon as half A is done
                psB = pp.tile([128, 2 * B], F32, tag="psB", name="psB")
                h_ode = wp.tile([128, 2 * B], B16, tag="h_ode", name="h_ode")
                for m in range(2):
                    sl = psB[:, m * B:(m + 1) * B]
                    ms = slice(m * 128, (m + 1) * 128)
                    mm(sl, w["ode1_w"][:, ms], ymb[:], start=True, stop=True)
                    nc.scalar.activation(h_ode[:, m * B:(m + 1) * B], sl, AF.Tanh,
                                         bias=w["ode1_bc"][:, m:m + 1])

                # ODE out and mask colsum broadcast share one bank
                psF = pp.tile([128, 2 * B], F32, tag="psF", name="psF")
                mm(psF[:, 0:B], w["ode2_k0"][:], h_ode[:, 0:B], start=True, stop=False)
                mm(psF[:, 0:B], w["ode2_k1"][:], h_ode[:, B:], start=False, stop=True)

                # T = dt*(ode_out + b2) via ACT scale/bias columns;
                # Yode = Ym + T (fp32) plus a bf16 copy for the PE
                t_ode = wp.tile([L, B], F32, tag="t_ode", name="t_ode")
                nc.scalar.activation(t_ode[:], psF[:, 0:B], AF.Identity,
                                     bias=w["b2dt"][:, t:t + 1],
                                     scale=w["dt_b"][:, t:t + 1])
                if False:  # STT alternative to the ACT-scale dt path
                    yode = wp.tile([L, B], F32, tag="yode", name="yode")
                    nc.vector.scalar_tensor_tensor(
                        yode[:], psF[:, 0:B], dt_col := w["dt_b"][:, t:t + 1],
                        ym[cur][:], op0=OP.mult, op1=OP.add)
                else:
                    yode = wp.tile([L, B], F32, tag="yode", name="yode")
                    nc.vector.tensor_tensor(yode[:], t_ode[:], ym[cur][:], op=OP.add)
                yodeb = wp.tile([L, B], B16, tag="yodeb", name="yodeb")
                nc.vector.tensor_copy(yodeb[:], yode[:])

                # update+reset gate layer 1 (4 m-halves in one 2-bank tile);
                # k-tile order: x first (ready earliest), then ys, then yode
                psA = pp.tile([128, 4 * B], F32, tag="psA", name="psA")
                for gi, net in enumerate(("ug1", "rg1")):
                    for m in range(2):
                        sl = psA[:, (2 * gi + m) * B:(2 * gi + m + 1) * B]
                        ms = slice(m * 128, (m + 1) * 128)
                        mm(sl, w[net + "_kx"][:, ms], xb[:], start=True, stop=False)
                        mm(sl, w[net + "_k1"][:, ms], ysb[:], start=False, stop=False)
                        mm(sl, w[net + "_k0"][:, ms], yodeb[:], start=False, stop=True)
                # mask colsum broadcast, after the kx matmuls (x-DMA wait seen)
                mm(psF[:, B:], msel[:], xb[:INPUT_DIM, :], start=True, stop=True)
                # tanh per gate so ug2 starts before the rg half finishes
                h_g1 = wp.tile([128, 4 * B], B16, tag="h_g1", name="h_g1")
                nc.scalar.activation(h_g1[:, 0:2 * B], psA[:, 0:2 * B], AF.Tanh)
                nc.scalar.activation(h_g1[:, 2 * B:], psA[:, 2 * B:], AF.Tanh)

                # gate layer 2: U | R pre-acts -> tanh(z/2) (+b/2 via bias col)
                psD = pp.tile([128, 2 * B], F32, tag="psD", name="psD")
                t_ur = wp.tile([128, 2 * B], B16, tag="t_ur", name="t_ur")
                for gi, net in enumerate(("ug2", "rg2")):
                    sl = psD[:, gi * B:(gi + 1) * B]
                    hbase = 2 * gi * B
                    mm(sl, w[net + "_k0"][:], h_g1[:, hbase:hbase + B], start=True, stop=False)
                    mm(sl, w[net + "_k1"][:], h_g1[:, hbase + B:hbase + 2 * B], start=False, stop=True)
                    nc.scalar.activation(t_ur[:, gi * B:(gi + 1) * B], sl, AF.Tanh,
                                         bias=w[net + "_bc"][:, 0:1], scale=0.5)

                # reset-gated state (carries factor 2; ns1 k0/k1 pre-scaled 0.5)
                am2 = wp.tile([L, B], B16, tag="am2", name="am2")
                nc.vector.scalar_tensor_tensor(
                    am2[:], t_ur[:, B:], 1.0, yode[:], op0=OP.add, op1=OP.mult)
                as2 = wp.tile([L, B], B16, tag="as2", name="as2")
                nc.vector.scalar_tensor_tensor(
                    as2[:], t_ur[:, B:], 1.0, ys[cur][:], op0=OP.add, op1=OP.mult)

                # new-state layer 1
                psC = pp.tile([128, 2 * B], F32, tag="psC", name="psC")
                for m in range(2):
                    sl = psC[:, m * B:(m + 1) * B]
                    ms = slice(m * 128, (m + 1) * 128)
                    mm(sl, w["ns1_kx"][:, ms], xb[:], start=True, stop=False)
                    mm(sl, w["ns1_k0"][:, ms], am2[:], start=False, stop=False)
                    mm(sl, w["ns1_k1"][:, ms], as2[:], start=False, stop=True)
                h_ns = wp.tile([128, 2 * B], B16, tag="h_ns", name="h_ns")
                nc.scalar.activation(h_ns[:, 0:B], psC[:, 0:B], AF.Tanh)
                nc.scalar.activation(h_ns[:, B:], psC[:, B:], AF.Tanh)

                # new-state layer 2: NM | NS pre-acts.  The NM half also
                # accumulates (+bm - Yode) so the gate blend reads PSUM once.
                psE = pp.tile([128, 2 * B], F32, tag="psE", name="psE")
                for m in range(2):
                    sl = psE[:, m * B:(m + 1) * B]
                    ms = slice(m * 128, (m + 1) * 128)
                    mm(sl, w["ns2_k0"][:, ms], h_ns[:, 0:B], start=True, stop=False)
                    mm(sl, w["ns2_k1"][:, ms], h_ns[:, B:], start=False,
                       stop=(m == 1))
                    if m == 0:
                        # fold (+bm - Yode) into the NM half so the gate
                        # blend can read PSUM directly (one DVE op saved)
                        mm(sl, w["ns2_bm16"][:], ones_row[:], start=False, stop=False)
                        mm(sl, w["neg_eye"][:], yodeb[:], start=False, stop=True)

                # G = 0.5*m*(1 - T_u)
                t1 = wp.tile([L, B], F32, tag="t1", name="t1")
                nc.vector.tensor_scalar(t1[:], t_ur[:, 0:B], -0.5, 0.5,
                                        op0=OP.mult, op1=OP.add)
                g = wp.tile([L, B], F32, tag="g", name="g")
                nc.vector.scalar_tensor_tensor(
                    g[:], psF[:, B:], 0.0, t1[:], op0=OP.is_gt, op1=OP.mult)

                # mean channel: Ym' = Yode + G*(NM + bm - Yode)
                pm = wp.tile([L, B], F32, tag="pm", name="pm")
                nc.vector.tensor_tensor(pm[:], g[:], psE[:, 0:B], op=OP.mult)
                nc.vector.tensor_tensor(ym[nxt][:], yode[:], pm[:], op=OP.add)
                nc.vector.tensor_copy(ymb[:], ym[nxt][:])

                # std channel: softplus(x)=log1p(e^x) via Newton, then gate
                e_t = wp.tile([L, B], F32, tag="e_t", name="e_t")
                nc.scalar.activation(e_t[:], psE[:, B:], AF.Exp,
                                     bias=w["ns2_bs"][:, 0:1])
                xa = wp.tile([L, B], F32, tag="xa", name="xa")
                nc.scalar.activation(xa[:], psE[:, B:], AF.Abs,
                                     bias=w["ns2_bs"][:, 0:1])
                wx = wp.tile([L, B], B16, tag="wx", name="wx")
                nc.scalar.activation(wx[:], xa[:], AF.Exp, scale=-1.0)
                rl = wp.tile([L, B], F32, tag="rl", name="rl")
                nc.vector.tensor_scalar(rl[:], psE[:, B:], w["ns2_bs"][:, 0:1],
                                        0.0, op0=OP.add, op1=OP.max)
                a_t = wp.tile([L, B], F32, tag="a_t", name="a_t")
                nc.vector.tensor_scalar(a_t[:], e_t[:], 1.0, None, op0=OP.add)
                y_sp = wp.tile([L, B], F32, tag="ysp0", name="ysp0")
                nc.vector.scalar_tensor_tensor(
                    y_sp[:], wx[:], LN2, rl[:], op0=OP.mult, op1=OP.add)
                for it in range(sp_iters):
                    u_t = wp.tile([L, B], F32, tag=f"usp{it}", name=f"usp{it}")
                    nc.scalar.activation(u_t[:], y_sp[:], AF.Exp, scale=-1.0)
                    tt = wp.tile([L, B], F32, tag=f"tsp{it}", name=f"tsp{it}")
                    nc.vector.tensor_tensor(tt[:], a_t[:], u_t[:], op=OP.mult)
                    ts_ = wp.tile([L, B], F32, tag=f"tss{it}", name=f"tss{it}")
                    nc.vector.tensor_tensor(ts_[:], tt[:], y_sp[:], op=OP.add)
                    y_new = wp.tile([L, B], F32, tag=f"ysp{it + 1}", name=f"ysp{it + 1}")
                    nc.vector.tensor_scalar(y_new[:], ts_[:], -1.0, None, op0=OP.add)
                    y_sp = y_new
                ds = wp.tile([L, B], F32, tag="ds", name="ds")
                nc.vector.scalar_tensor_tensor(
                    ds[:], y_sp[:], 1e-6, ys[cur][:], op0=OP.add, op1=OP.subtract)
                ps_ = wp.tile([L, B], F32, tag="ps_", name="ps_")
                nc.vector.tensor_tensor(ps_[:], g[:], ds[:], op=OP.mult)
                nc.vector.tensor_tensor(ys[nxt][:], ys[cur][:], ps_[:], op=OP.add)
                nc.vector.tensor_copy(ysb[:], ys[nxt][:])

            # ---- final transform ----
            fin = n_tp % 2
            psB = pp.tile([128, 2 * B], F32, tag="psB", name="psB")
            for m in range(2):
                sl = psB[:, m * B:(m + 1) * B]
                ms = slice(m * 128, (m + 1) * 128)
                mm(sl, w["tz1_b"][:, ms], ones_row[:], start=True, stop=False)
                mm(sl, w["tz1_k0"][:, ms], ymb[:], start=False, stop=False)
                mm(sl, w["tz1_k1"][:, ms], ysb[:], start=False, stop=True)
            h_tz = wp.tile([128, 2 * B], B16, tag="h_ode", name="h_tz")
            nc.scalar.activation(h_tz[:], psB[:], AF.Tanh)
            psE = pp.tile([128, 2 * B], F32, tag="psE", name="psE2")
            for m in range(2):
                sl = psE[:, m * B:(m + 1) * B]
                ms = slice(m * 128, (m + 1) * 128)
                mm(sl, w["tz2_k0"][:, ms], h_tz[:, 0:B], start=True, stop=False)
                mm(sl, w["tz2_k1"][:, ms], h_tz[:, B:], start=False, stop=True)
            o_m = wp.tile([L, B], F32, tag="o_m", name="o_m")
            nc.scalar.activation(o_m[:], psE[:, 0:B], AF.Identity,
                                 bias=w["tz2_bm"][:, 0:1])
            o_s = wp.tile([L, B], F32, tag="o_s", name="o_s")
            nc.scalar.activation(o_s[:], psE[:, B:], AF.Abs,
                                 bias=w["tz2_bs"][:, 0:1])
            nc.sync.dma_start(d_om[:], o_m[:])
            nc.sync.dma_start(d_os[:], o_s[:])

    nc.compile()
    return nc


# --------------------------------------------------------------------------
# host-side packing
# --------------------------------------------------------------------------
def _prep_in_maps(inputs, n_tp):
    F = np.float32
    d = {k: np.ascontiguousarray(np.asarray(v, F)) for k, v in inputs.items()}
    obs = d["obs_tps"][:n_tp]
    data = d["data"][:, :n_tp]

    dd = (obs[:-1] - obs[1:])[::-1]
    dts = np.concatenate([np.full((1,), -0.01, F), dd])
    dt_b = np.ascontiguousarray(np.broadcast_to(dts[None, :], (128, n_tp)))
    b2dt = np.ascontiguousarray(d["ode_b2"][:, None] * dts[None, :])

    # [t, c, subj], reversed in time, bf16
    x_rev = np.ascontiguousarray(data.transpose(1, 2, 0)[::-1]).astype(BF)

    ns_w1s = d["ns_w1"].copy()
    ns_w1s[:2 * L] *= F(0.5)

    def kx(w1, b1):
        return np.vstack([w1[2 * L:], b1[None, :]])

    bf = {
        "ug1_k0": d["ug_w1"][:L], "ug1_k1": d["ug_w1"][L:2 * L],
        "ug1_kx": kx(d["ug_w1"], d["ug_b1"]),
        "rg1_k0": d["rg_w1"][:L], "rg1_k1": d["rg_w1"][L:2 * L],
        "rg1_kx": kx(d["rg_w1"], d["rg_b1"]),
        "ns1_k0": ns_w1s[:L], "ns1_k1": ns_w1s[L:2 * L],
        "ns1_kx": kx(d["ns_w1"], d["ns_b1"]),
        "ode1_w": d["ode_w1"],
        "ode2_k0": d["ode_w2"][:128], "ode2_k1": d["ode_w2"][128:],
        "ug2_k0": d["ug_w2"][:128], "ug2_k1": d["ug_w2"][128:],
        "rg2_k0": d["rg_w2"][:128], "rg2_k1": d["rg_w2"][128:],
        "ns2_k0": d["ns_w2"][:128], "ns2_k1": d["ns_w2"][128:],
        "ns2_bm16": d["ns_b2"][None, :L],
        "neg_eye": -np.eye(L, dtype=F),
        "tz1_k0": d["tz_w1"][:L], "tz1_k1": d["tz_w1"][L:],
        "tz1_b": d["tz_b1"][None, :],
        "tz2_k0": d["tz_w2"][:128], "tz2_k1": d["tz_w2"][128:],
    }
    shared = {k: np.ascontiguousarray(v.astype(BF)) for k, v in bf.items()}
    shared["dt_b"] = dt_b
    shared["b2dt"] = b2dt
    shared["ode1_bc"] = np.ascontiguousarray(d["ode_b1"].reshape(2, 128).T)
    shared["ug2_bc"] = np.ascontiguousarray(d["ug_b2"][:, None] * F(0.5))
    shared["rg2_bc"] = np.ascontiguousarray(d["rg_b2"][:, None] * F(0.5))
    shared["ns2_bm"] = np.ascontiguousarray(d["ns_b2"][:L, None])
    shared["ns2_bs"] = np.ascontiguousarray(d["ns_b2"][L:, None])
    shared["tz2_bm"] = np.ascontiguousarray(d["tz_b2"][:L, None])
    shared["tz2_bs"] = np.ascontiguousarray(d["tz_b2"][L:, None])

    in_maps = []
    for c in range(N_CORES):
        m = dict(shared)
        m["x_rev"] = np.ascontiguousarray(x_rev[:, :, c * B:(c + 1) * B])
        in_maps.append(m)
    return in_maps


def kernel(**inputs):
    from concourse.bass_utils import run_bass_kernel_spmd

    key = (N_TP, SP_ITERS)
    if key not in _CACHE:
        _CACHE[key] = _build(*key)
    nc = _CACHE[key]

    in_maps = _prep_in_maps(inputs, N_TP)
    res = run_bass_kernel_spmd(nc, in_maps, list(range(N_CORES)))
    outs = res.results

    mean = np.empty((1, N_SUBJ, L), np.float32)
    std = np.empty((1, N_SUBJ, L), np.float32)
    for c in range(N_CORES):
        mean[0, c * B:(c + 1) * B] = outs[c]["out_m"].T
        std[0, c * B:(c + 1) * B] = outs[c]["out_s"].T
    return mean, std


# revision 33
# speedup vs baseline: 2.5280x; 1.0735x over previous
"""Trainium2 Bass kernel for an ODE-RNN encoder (z0 posterior).

Model: 128-step reversed-time GRU-like recurrence with an Euler ODE step on
the mean channel, then a final transform producing (mean_z0, std_z0).

Strategy: data-parallel over the subject (batch) dim across 8 NeuronCores,
weights replicated.  Everything runs on-chip in a transposed layout
([feature, batch], batch=256 on the free dim).  Key points:
- matmul operands are bf16 (fp32 PSUM accumulate): fp32 matmuls on TRN2
  lower to TWO half-speed PE passes, bf16 is single-pass with fast weight
  load.  State tensors stay fp32; small bf16 copies feed the PE.
  (host-validated: ~5e-3 max rel error vs the fp32 reference)
- all biases ride either a ones-row appended to the streamed x tile
  (layer-1 nets) or ACT per-partition bias vectors — zero per-step bias
  matmuls;  Yode = Ym + dt*(ode_out + b2) becomes one ACT op with
  per-partition scale=dt and bias=dt*b2 (host-precomputed per step);
- sigmoid(z) = 0.5 + 0.5*tanh(z/2) keeps every transcendental in the
  resident `exp_and_others` ACT table set (no per-step table switches);
- softplus(x) = log1p(exp(x)) via Newton on y: e^y = 1 + e^x, seeded with
  y0 = relu(x) + ln2*exp(-|x|) (1 iteration: ~1.2e-3, under the bf16 floor);
- the update gate, the observation mask (broadcast via an all-ones K=64
  selector matmul) and the GRU convex combination fold into one factor
  G = 0.5*m*(1-tanh(zU/2)):  new = old + G*(cand - old); the (cand - old)
  part is accumulated in PSUM via a negative-identity matmul;
- TRN2 allows ONE sync wait per instruction; Bacc legalizes the rest, but
  K=1 dummy matmuls + accumulation-group ordering keep the PE free of
  multi-wait event-semaphore preambles in the steady state.
"""
import sys
import numpy as np
import ml_dtypes

for _p in ("/opt/trn_rl_repo", "/root/.axon_site/_ro/trn_rl_repo"):
    if _p not in sys.path:
        sys.path.append(_p)

N_SUBJ, N_TP, INPUT_DIM, LATENT, N_UNIT = 2048, 128, 64, 128, 256
HALF = INPUT_DIM // 2
N_CORES = 8
B = N_SUBJ // N_CORES          # 256 subjects per core (free dim)
L = LATENT
SP_ITERS = 1                   # softplus Newton refinements
LN2 = float(np.log(2.0))
BF = ml_dtypes.bfloat16

_CACHE = {}


# --------------------------------------------------------------------------
# Bass program
# --------------------------------------------------------------------------
def _build(n_tp, sp_iters):
    import concourse.mybir as mybir
    from concourse import bacc, tile

    F32 = mybir.dt.float32
    B16 = mybir.dt.bfloat16
    AF = mybir.ActivationFunctionType
    OP = mybir.AluOpType

    # Bacc (not plain Bass): its compile() legalizes the TRN2 one-sync-wait-
    # per-instruction limit (event-semaphore splitting, matmul-wait moves).
    nc = bacc.Bacc(None)

    # ---- DRAM I/O ----
    d_x = nc.dram_tensor("x_rev", [n_tp, INPUT_DIM, B], B16, kind="ExternalInput")

    bspec = {  # bf16 weights (matmul operands)
        "ug1_k0": [L, N_UNIT], "ug1_k1": [L, N_UNIT], "ug1_kx": [INPUT_DIM + 1, N_UNIT],
        "rg1_k0": [L, N_UNIT], "rg1_k1": [L, N_UNIT], "rg1_kx": [INPUT_DIM + 1, N_UNIT],
        "ns1_k0": [L, N_UNIT], "ns1_k1": [L, N_UNIT], "ns1_kx": [INPUT_DIM + 1, N_UNIT],
        "ode1_w": [L, N_UNIT],
        "ode2_k0": [128, L], "ode2_k1": [128, L],
        "ug2_k0": [128, L], "ug2_k1": [128, L],
        "rg2_k0": [128, L], "rg2_k1": [128, L],
        "ns2_k0": [128, 2 * L], "ns2_k1": [128, 2 * L], "ns2_bm16": [1, L],
        "neg_eye": [L, L],
        "tz1_k0": [L, N_UNIT], "tz1_k1": [L, N_UNIT], "tz1_b": [1, N_UNIT],
        "tz2_k0": [128, 2 * L], "tz2_k1": [128, 2 * L],
    }
    fspec = {  # fp32 per-partition columns (ACT bias/scale, STT scalar APs)
        "ode1_bc": [128, 2], "ug2_bc": [128, 1], "rg2_bc": [128, 1],
        "ns2_bm": [128, 1], "ns2_bs": [128, 1], "tz2_bm": [128, 1], "tz2_bs": [128, 1],
        "dt_b": [128, n_tp], "b2dt": [128, n_tp],
    }
    d_w = {k: nc.dram_tensor(k, v, B16, kind="ExternalInput") for k, v in bspec.items()}
    d_w.update({k: nc.dram_tensor(k, v, F32, kind="ExternalInput")
                for k, v in fspec.items()})

    d_om = nc.dram_tensor("out_m", [L, B], F32, kind="ExternalOutput")
    d_os = nc.dram_tensor("out_s", [L, B], F32, kind="ExternalOutput")

    with tile.TileContext(nc) as tc:
        with (
            tc.tile_pool(name="const", bufs=1) as cp,
            tc.tile_pool(name="work", bufs=3) as wp,
            tc.tile_pool(name="ps", bufs=1, space="PSUM") as pp,
        ):
            # ---- resident constants / weights ----
            w = {}
            for k, shp in bspec.items():
                w[k] = cp.tile(shp, B16, tag=k, name=k)
                nc.sync.dma_start(w[k][:], d_w[k][:])
            for k, shp in fspec.items():
                w[k] = cp.tile(shp, F32, tag=k, name=k)
                nc.sync.dma_start(w[k][:], d_w[k][:])
            ones_row = cp.tile([1, B], B16, tag="ones_row", name="ones_row")
            nc.vector.memset(ones_row[:], 1.0)
            # mask-channel selector: zeros over value rows, ones over mask rows
            msel = cp.tile([INPUT_DIM, 128], B16, tag="msel", name="msel")
            nc.vector.memset(msel[:HALF, :], 0.0)
            nc.vector.memset(msel[HALF:, :], 1.0)

            xbufs = []
            for j in range(3):
                xb = cp.tile([INPUT_DIM + 1, B], B16, tag=f"xb{j}", name=f"xb{j}")
                nc.vector.memset(xb[INPUT_DIM:, :], 1.0)
                xbufs.append(xb)

            ym = [cp.tile([L, B], F32, tag=f"ym{i}", name=f"ym{i}") for i in range(2)]
            ys = [cp.tile([L, B], F32, tag=f"ys{i}", name=f"ys{i}") for i in range(2)]
            ymb = cp.tile([L, B], B16, tag="ymb", name="ymb")
            ysb = cp.tile([L, B], B16, tag="ysb", name="ysb")
            nc.vector.memset(ym[0][:], 0.0)
            nc.vector.memset(ys[0][:], 0.0)
            nc.vector.memset(ymb[:], 0.0)
            nc.vector.memset(ysb[:], 0.0)

            mm = nc.tensor.matmul

            # Warm the PE's clock past every weight DMA with K=1 dummy
            # matmuls so steady-state matmuls only wait on one producer.
            scr = pp.tile([1, 16], F32, tag="scr", name="scr")
            for k in bspec:
                mm(scr[0:1, 0:1], w[k][0:1, 0:1], w[k][0:1, 1:2],
                   start=True, stop=True)
            # DVE/ACT read fp32 DMA-produced columns: warm those clocks too
            nf = len(fspec)
            warm_dv = cp.tile([1, 2 * nf], F32, tag="warm_dv", name="warm_dv")
            for j, k in enumerate(fspec):
                nc.vector.tensor_copy(warm_dv[0:1, j:j + 1], w[k][0:1, 0:1])
                nc.scalar.copy(warm_dv[0:1, nf + j:nf + j + 1], w[k][0:1, 0:1])

            # ---- the recurrence ----
            for t in range(n_tp):
                cur, nxt = t % 2, (t + 1) % 2
                xb = xbufs[t % 3]
                nc.sync.dma_start(xb[:INPUT_DIM, :], d_x[t])
                # absorb the x-DMA wait into a K=1 dummy
                mm(scr[0:1, 0:1], xb[0:1, 0:1], xb[0:1, 1:2], start=True, stop=True)

                # ODE hidden: tanh(ode_w1^T @ Ym + b1); split per m-half so
                # the ode2 k0 matmul starts as soon as half A is done
                psB = pp.tile([128, 2 * B], F32, tag="psB", name="psB")
                h_ode = wp.tile([128, 2 * B], B16, tag="h_ode", name="h_ode")
                for m in range(2):
                    sl = psB[:, m * B:(m + 1) * B]
                    ms = slice(m * 128, (m + 1) * 128)
                    mm(sl, w["ode1_w"][:, ms], ymb[:], start=True, stop=True)
                    nc.scalar.activation(h_ode[:, m * B:(m + 1) * B], sl, AF.Tanh,
                                         bias=w["ode1_bc"][:, m:m + 1])

                # ODE out and mask colsum broadcast share one bank
                psF = pp.tile([128, 2 * B], F32, tag="psF", name="psF")
                mm(psF[:, 0:B], w["ode2_k0"][:], h_ode[:, 0:B], start=True, stop=False)
                mm(psF[:, 0:B], w["ode2_k1"][:], h_ode[:, B:], start=False, stop=True)

                # T = dt*(ode_out + b2) via ACT scale/bias columns;
                # Yode = Ym + T (fp32) plus a bf16 copy for the PE
                t_ode = wp.tile([L, B], F32, tag="t_ode", name="t_ode")
                nc.scalar.activation(t_ode[:], psF[:, 0:B], AF.Identity,
                                     bias=w["b2dt"][:, t:t + 1],
                                     scale=w["dt_b"][:, t:t + 1])
                if False:  # STT alternative to the ACT-scale dt path
                    yode = wp.tile([L, B], F32, tag="yode", name="yode")
                    nc.vector.scalar_tensor_tensor(
                        yode[:], psF[:, 0:B], dt_col := w["dt_b"][:, t:t + 1],
                        ym[cur][:], op0=OP.mult, op1=OP.add)
                else:
                    yode = wp.tile([L, B], F32, tag="yode", name="yode")
                    nc.vector.tensor_tensor(yode[:], t_ode[:], ym[cur][:], op=OP.add)
                yodeb = wp.tile([L, B], B16, tag="yodeb", name="yodeb")
                nc.vector.tensor_copy(yodeb[:], yode[:])

                # absorb the ys-producer wait here (not at step start: the PE
                # queue is in-order, and an early dummy would stall the whole
                # ODE path behind the std-channel tail of the previous step)
                mm(scr[0:1, 1:2], ysb[0:1, 0:1], ysb[0:1, 1:2], start=True, stop=True)
                # update+reset gate layer 1 (4 m-halves in one 2-bank tile);
                # k-tile order: x first (ready earliest), then ys, then yode
                psA = pp.tile([128, 4 * B], F32, tag="psA", name="psA")
                # rg (reset gate) first throughout: the critical chain runs
                # through R -> am2/as2 -> ns1; U is only needed at the final
                # gate blend
                for gi, net in ((1, "rg1"), (0, "ug1")):
                    for m in range(2):
                        sl = psA[:, (2 * gi + m) * B:(2 * gi + m + 1) * B]
                        ms = slice(m * 128, (m + 1) * 128)
                        mm(sl, w[net + "_kx"][:, ms], xb[:], start=True, stop=False)
                        mm(sl, w[net + "_k1"][:, ms], ysb[:], start=False, stop=False)
                        mm(sl, w[net + "_k0"][:, ms], yodeb[:], start=False, stop=True)
                # mask colsum broadcast, after the kx matmuls (x-DMA wait seen)
                mm(psF[:, B:], msel[:], xb[:INPUT_DIM, :], start=True, stop=True)
                # tanh per gate so rg2 starts before the ug half finishes
                h_g1 = wp.tile([128, 4 * B], B16, tag="h_g1", name="h_g1")
                nc.scalar.activation(h_g1[:, 2 * B:], psA[:, 2 * B:], AF.Tanh)
                nc.scalar.activation(h_g1[:, 0:2 * B], psA[:, 0:2 * B], AF.Tanh)

                # gate layer 2: U | R pre-acts -> tanh(z/2) (+b/2 via bias col)
                psD = pp.tile([128, 2 * B], F32, tag="psD", name="psD")
                t_ur = wp.tile([128, 2 * B], B16, tag="t_ur", name="t_ur")
                for gi, net in ((1, "rg2"), (0, "ug2")):
                    sl = psD[:, gi * B:(gi + 1) * B]
                    hbase = 2 * gi * B
                    mm(sl, w[net + "_k0"][:], h_g1[:, hbase:hbase + B], start=True, stop=False)
                    mm(sl, w[net + "_k1"][:], h_g1[:, hbase + B:hbase + 2 * B], start=False, stop=True)
                    nc.scalar.activation(t_ur[:, gi * B:(gi + 1) * B], sl, AF.Tanh,
                                         bias=w[net + "_bc"][:, 0:1], scale=0.5)

                # reset-gated state (carries factor 2; ns1 k0/k1 pre-scaled 0.5)
                am2 = wp.tile([L, B], B16, tag="am2", name="am2")
                nc.vector.scalar_tensor_tensor(
                    am2[:], t_ur[:, B:], 1.0, yode[:], op0=OP.add, op1=OP.mult)
                as2 = wp.tile([L, B], B16, tag="as2", name="as2")
                nc.vector.scalar_tensor_tensor(
                    as2[:], t_ur[:, B:], 1.0, ys[cur][:], op0=OP.add, op1=OP.mult)

                # new-state layer 1
                psC = pp.tile([128, 2 * B], F32, tag="psC", name="psC")
                for m in range(2):
                    sl = psC[:, m * B:(m + 1) * B]
                    ms = slice(m * 128, (m + 1) * 128)
                    mm(sl, w["ns1_kx"][:, ms], xb[:], start=True, stop=False)
                    mm(sl, w["ns1_k0"][:, ms], am2[:], start=False, stop=False)
                    mm(sl, w["ns1_k1"][:, ms], as2[:], start=False, stop=True)
                h_ns = wp.tile([128, 2 * B], B16, tag="h_ns", name="h_ns")
                nc.scalar.activation(h_ns[:, 0:B], psC[:, 0:B], AF.Tanh)
                nc.scalar.activation(h_ns[:, B:], psC[:, B:], AF.Tanh)

                # new-state layer 2: NM | NS pre-acts.  The NM half also
                # accumulates (+bm - Yode) so the gate blend reads PSUM once.
                psE = pp.tile([128, 2 * B], F32, tag="psE", name="psE")
                for m in range(2):
                    sl = psE[:, m * B:(m + 1) * B]
                    ms = slice(m * 128, (m + 1) * 128)
                    mm(sl, w["ns2_k0"][:, ms], h_ns[:, 0:B], start=True, stop=False)
                    mm(sl, w["ns2_k1"][:, ms], h_ns[:, B:], start=False,
                       stop=(m == 1))
                    if m == 0:
                        # fold (+bm - Yode) into the NM half so the gate
                        # blend can read PSUM directly (one DVE op saved)
                        mm(sl, w["ns2_bm16"][:], ones_row[:], start=False, stop=False)
                        mm(sl, w["neg_eye"][:], yodeb[:], start=False, stop=True)

                # G = 0.5*m*(1 - T_u)
                t1 = wp.tile([L, B], F32, tag="t1", name="t1")
                nc.vector.tensor_scalar(t1[:], t_ur[:, 0:B], -0.5, 0.5,
                                        op0=OP.mult, op1=OP.add)
                g = wp.tile([L, B], F32, tag="g", name="g")
                nc.vector.scalar_tensor_tensor(
                    g[:], psF[:, B:], 0.0, t1[:], op0=OP.is_gt, op1=OP.mult)

                # mean channel: Ym' = Yode + G*(NM + bm - Yode)
                pm = wp.tile([L, B], F32, tag="pm", name="pm")
                nc.vector.tensor_tensor(pm[:], g[:], psE[:, 0:B], op=OP.mult)
                nc.vector.tensor_tensor(ym[nxt][:], yode[:], pm[:], op=OP.add)
                nc.vector.tensor_copy(ymb[:], ym[nxt][:])

                # std channel: softplus(x)=log1p(e^x) via Newton, then gate
                e_t = wp.tile([L, B], F32, tag="e_t", name="e_t")
                nc.scalar.activation(e_t[:], psE[:, B:], AF.Exp,
                                     bias=w["ns2_bs"][:, 0:1])
                xa = wp.tile([L, B], F32, tag="xa", name="xa")
                nc.scalar.activation(xa[:], psE[:, B:], AF.Abs,
                                     bias=w["ns2_bs"][:, 0:1])
                wx = wp.tile([L, B], B16, tag="wx", name="wx")
                nc.scalar.activation(wx[:], xa[:], AF.Exp, scale=-1.0)
                rl = wp.tile([L, B], F32, tag="rl", name="rl")
                nc.vector.tensor_scalar(rl[:], psE[:, B:], w["ns2_bs"][:, 0:1],
                                        0.0, op0=OP.add, op1=OP.max)
                a_t = wp.tile([L, B], F32, tag="a_t", name="a_t")
                nc.vector.tensor_scalar(a_t[:], e_t[:], 1.0, None, op0=OP.add)
                y_sp = wp.tile([L, B], F32, tag="ysp0", name="ysp0")
                nc.vector.scalar_tensor_tensor(
                    y_sp[:], wx[:], LN2, rl[:], op0=OP.mult, op1=OP.add)
                for it in range(sp_iters):
                    u_t = wp.tile([L, B], F32, tag=f"usp{it}", name=f"usp{it}")
                    nc.scalar.activation(u_t[:], y_sp[:], AF.Exp, scale=-1.0)
                    tt = wp.tile([L, B], F32, tag=f"tsp{it}", name=f"tsp{it}")
                    nc.vector.tensor_tensor(tt[:], a_t[:], u_t[:], op=OP.mult)
                    ts_ = wp.tile([L, B], F32, tag=f"tss{it}", name=f"tss{it}")
                    nc.vector.tensor_tensor(ts_[:], tt[:], y_sp[:], op=OP.add)
                    y_new = wp.tile([L, B], F32, tag=f"ysp{it + 1}", name=f"ysp{it + 1}")
                    nc.vector.tensor_scalar(y_new[:], ts_[:], -1.0, None, op0=OP.add)
                    y_sp = y_new
                ds = wp.tile([L, B], F32, tag="ds", name="ds")
                nc.vector.scalar_tensor_tensor(
                    ds[:], y_sp[:], 1e-6, ys[cur][:], op0=OP.add, op1=OP.subtract)
                ps_ = wp.tile([L, B], F32, tag="ps_", name="ps_")
                nc.vector.tensor_tensor(ps_[:], g[:], ds[:], op=OP.mult)
                nc.vector.tensor_tensor(ys[nxt][:], ys[cur][:], ps_[:], op=OP.add)
                nc.vector.tensor_copy(ysb[:], ys[nxt][:])

            # ---- final transform ----
            fin = n_tp % 2
            psB = pp.tile([128, 2 * B], F32, tag="psB", name="psB")
            for m in range(2):
                sl = psB[:, m * B:(m + 1) * B]
                ms = slice(m * 128, (m + 1) * 128)
                mm(sl, w["tz1_b"][:, ms], ones_row[:], start=True, stop=False)
                mm(sl, w["tz1_k0"][:, ms], ymb[:], start=False, stop=False)
                mm(sl, w["tz1_k1"][:, ms], ysb[:], start=False, stop=True)
            h_tz = wp.tile([128, 2 * B], B16, tag="h_ode", name="h_tz")
            nc.scalar.activation(h_tz[:], psB[:], AF.Tanh)
            psE = pp.tile([128, 2 * B], F32, tag="psE", name="psE2")
            for m in range(2):
                sl = psE[:, m * B:(m + 1) * B]
                ms = slice(m * 128, (m + 1) * 128)
                mm(sl, w["tz2_k0"][:, ms], h_tz[:, 0:B], start=True, stop=False)
                mm(sl, w["tz2_k1"][:, ms], h_tz[:, B:], start=False, stop=True)
            o_m = wp.tile([L, B], F32, tag="o_m", name="o_m")
            nc.scalar.activation(o_m[:], psE[:, 0:B], AF.Identity,
                                 bias=w["tz2_bm"][:, 0:1])
            o_s = wp.tile([L, B], F32, tag="o_s", name="o_s")
            nc.scalar.activation(o_s[:], psE[:, B:], AF.Abs,
                                 bias=w["tz2_bs"][:, 0:1])
            nc.sync.dma_start(d_om[:], o_m[:])
            nc.sync.dma_start(d_os[:], o_s[:])

    nc.compile()
    return nc


# --------------------------------------------------------------------------
# host-side packing
# --------------------------------------------------------------------------
def _prep_in_maps(inputs, n_tp):
    F = np.float32
    d = {k: np.ascontiguousarray(np.asarray(v, F)) for k, v in inputs.items()}
    obs = d["obs_tps"][:n_tp]
    data = d["data"][:, :n_tp]

    dd = (obs[:-1] - obs[1:])[::-1]
    dts = np.concatenate([np.full((1,), -0.01, F), dd])
    dt_b = np.ascontiguousarray(np.broadcast_to(dts[None, :], (128, n_tp)))
    b2dt = np.ascontiguousarray(d["ode_b2"][:, None] * dts[None, :])

    # [t, c, subj], reversed in time, bf16
    x_rev = np.ascontiguousarray(data.transpose(1, 2, 0)[::-1]).astype(BF)

    ns_w1s = d["ns_w1"].copy()
    ns_w1s[:2 * L] *= F(0.5)

    def kx(w1, b1):
        return np.vstack([w1[2 * L:], b1[None, :]])

    bf = {
        "ug1_k0": d["ug_w1"][:L], "ug1_k1": d["ug_w1"][L:2 * L],
        "ug1_kx": kx(d["ug_w1"], d["ug_b1"]),
        "rg1_k0": d["rg_w1"][:L], "rg1_k1": d["rg_w1"][L:2 * L],
        "rg1_kx": kx(d["rg_w1"], d["rg_b1"]),
        "ns1_k0": ns_w1s[:L], "ns1_k1": ns_w1s[L:2 * L],
        "ns1_kx": kx(d["ns_w1"], d["ns_b1"]),
        "ode1_w": d["ode_w1"],
        "ode2_k0": d["ode_w2"][:128], "ode2_k1": d["ode_w2"][128:],
        "ug2_k0": d["ug_w2"][:128], "ug2_k1": d["ug_w2"][128:],
        "rg2_k0": d["rg_w2"][:128], "rg2_k1": d["rg_w2"][128:],
        "ns2_k0": d["ns_w2"][:128], "ns2_k1": d["ns_w2"][128:],
        "ns2_bm16": d["ns_b2"][None, :L],
        "neg_eye": -np.eye(L, dtype=F),
        "tz1_k0": d["tz_w1"][:L], "tz1_k1": d["tz_w1"][L:],
        "tz1_b": d["tz_b1"][None, :],
        "tz2_k0": d["tz_w2"][:128], "tz2_k1": d["tz_w2"][128:],
    }
    shared = {k: np.ascontiguousarray(v.astype(BF)) for k, v in bf.items()}
    shared["dt_b"] = dt_b
    shared["b2dt"] = b2dt
    shared["ode1_bc"] = np.ascontiguousarray(d["ode_b1"].reshape(2, 128).T)
    shared["ug2_bc"] = np.ascontiguousarray(d["ug_b2"][:, None] * F(0.5))
    shared["rg2_bc"] = np.ascontiguousarray(d["rg_b2"][:, None] * F(0.5))
    shared["ns2_bm"] = np.ascontiguousarray(d["ns_b2"][:L, None])
    shared["ns2_bs"] = np.ascontiguousarray(d["ns_b2"][L:, None])
    shared["tz2_bm"] = np.ascontiguousarray(d["tz_b2"][:L, None])
    shared["tz2_bs"] = np.ascontiguousarray(d["tz_b2"][L:, None])

    in_maps = []
    for c in range(N_CORES):
        m = dict(shared)
        m["x_rev"] = np.ascontiguousarray(x_rev[:, :, c * B:(c + 1) * B])
        in_maps.append(m)
    return in_maps


def kernel(**inputs):
    from concourse.bass_utils import run_bass_kernel_spmd

    key = (N_TP, SP_ITERS)
    if key not in _CACHE:
        _CACHE[key] = _build(*key)
    nc = _CACHE[key]

    in_maps = _prep_in_maps(inputs, N_TP)
    res = run_bass_kernel_spmd(nc, in_maps, list(range(N_CORES)))
    outs = res.results

    mean = np.empty((1, N_SUBJ, L), np.float32)
    std = np.empty((1, N_SUBJ, L), np.float32)
    for c in range(N_CORES):
        mean[0, c * B:(c + 1) * B] = outs[c]["out_m"].T
        std[0, c * B:(c + 1) * B] = outs[c]["out_s"].T
    return mean, std
